# revision 5
# baseline (speedup 1.0000x reference)
"""Bass/Tile TRN2 kernel for nn_Attn: energies = einsum('sbh,bh->sb'), softmax over s,
output attn.T[:, None, :]  ([B, 1, S]).

Sharding: data-parallel over batch B=32 across 8 cores (4 batch elems per core).
Per-core compute:
  - 16 DMA loads of [128, 4*1024] f32 enc tiles (2 MiB each, 16 KiB contiguous per
    partition row) -- DMA-bound at ~358 GB/s per core.
  - For each (s-tile, b): one fused DVE tensor_tensor_reduce:
      prod = enc_tile[:, b] * hid_bcast[b];  energies_col = sum_h(prod)
    (single 1x pass over the data; the product itself lands in a dummy
    broadcast AP and is discarded).
  - energies grid [128, 64] -> PE transpose -> PSUM [64, 128] -> DMA rearrange
    to [4, 2048] (partition = batch).
  - softmax along free dim: reduce_max(negate) -> ACT exp(bias=-max) with fused
    accum sum -> reciprocal -> tensor_scalar mul -> DMA out [4, 1, 2048].
"""

import numpy as np

import concourse.tile as tile
import concourse.mybir as mybir
from concourse import bacc
from concourse.bass_utils import run_bass_kernel_spmd

S, B, H = 2048, 32, 1024
NCORES = 8
BL = B // NCORES  # 4 batch elems per core
PT = 128          # partition tile along s
NST = S // PT     # 16 s-tiles
FP32 = mybir.dt.float32

_CACHE = {}


def _build_body(tc, out, hid, enc, ident):
    nc = tc.nc
    enc_flat = enc.rearrange("s b h -> s (b h)")  # [S, BL*H]

    with (
        tc.tile_pool(name="const", bufs=1) as const_pool,
        tc.tile_pool(name="encp", bufs=3) as enc_pool,
        tc.tile_pool(name="psum", bufs=1, space="PSUM") as psum_pool,
    ):
        ident_sb = const_pool.tile([PT, PT], FP32)
        nc.sync.dma_start(ident_sb[:], ident)

        # hidden broadcast across all 128 partitions: [128, BL*H]
        hidb = const_pool.tile([PT, BL * H], FP32)
        nc.sync.dma_start(
            hidb[:],
            hid.rearrange("b h -> (b h)").unsqueeze(0).broadcast_to([PT, BL * H]),
        )

        # energies grid: grid[p, b*NST + st] = energies[st*128 + p, b]
        grid = const_pool.tile([PT, BL * NST], FP32)
        dummy = const_pool.tile([PT, 1], FP32)

        for st in range(NST):
            et = enc_pool.tile([PT, BL * H], FP32, tag="et")
            nc.sync.dma_start(et[:], enc_flat[st * PT:(st + 1) * PT, :])
            for b in range(BL):
                col = b * NST + st
                # fused multiply + free-dim sum in one DVE pass:
                # out = (et * 1.0) * hidb (discarded), accum = sum(out)
                nc.vector.scalar_tensor_tensor(
                    dummy[:].broadcast_to([PT, H]),
                    et[:, b * H:(b + 1) * H],
                    1.0,
                    hidb[:, b * H:(b + 1) * H],
                    op0=mybir.AluOpType.mult,
                    op1=mybir.AluOpType.mult,
                    accum_out=grid[:, col:col + 1],
                )

        # transpose grid -> [64, 128] in PSUM, then rearrange to [BL, S]
        gridT = psum_pool.tile([BL * NST, PT], FP32)
        nc.tensor.transpose(gridT[:], grid[:], ident_sb[:])

        gridT_sb = const_pool.tile([BL * NST, PT], FP32)
        nc.scalar.copy(gridT_sb[:], gridT[:])

        # Gather each batch's 16 partition rows of [128] into one [1, 2048] row.
        # (A single rearranged DMA can't cross partitions inside a free dim.)
        eT = const_pool.tile([BL, S], FP32)
        for b in range(BL):
            nc.sync.dma_start(eT[b:b + 1, :], gridT_sb[b * NST:(b + 1) * NST, :])

        # softmax over free dim (s), per-partition (b)
        negm = const_pool.tile([BL, 1], FP32)
        nc.vector.reduce_max(negm[:], eT[:], axis=mybir.AxisListType.X, negate=True)

        p_t = const_pool.tile([BL, S], FP32)
        ssum = const_pool.tile([BL, 1], FP32)
        nc.scalar.activation(
            p_t[:], eT[:], mybir.ActivationFunctionType.Exp,
            bias=negm[:], scale=1.0, accum_out=ssum[:],
        )

        rsum = const_pool.tile([BL, 1], FP32)
        nc.vector.reciprocal(rsum[:], ssum[:])

        attn = const_pool.tile([BL, S], FP32)
        nc.vector.tensor_scalar_mul(attn[:], p_t[:], rsum[:])

        nc.sync.dma_start(out.rearrange("b o s -> b (o s)"), attn[:])


def _build():
    if "nc" in _CACHE:
        return _CACHE["nc"]
    nc = bacc.Bacc(
        "TRN2",
        target_bir_lowering=False,
        debug=False,
        enable_asserts=False,
        num_devices=NCORES,
    )
    hid = nc.dram_tensor("hidden", [BL, H], FP32, kind="ExternalInput").ap()
    enc = nc.dram_tensor("encoder_outputs", [S, BL, H], FP32, kind="ExternalInput").ap()
    ident = nc.dram_tensor("identity", [PT, PT], FP32, kind="ExternalInput").ap()
    out = nc.dram_tensor("out", [BL, 1, S], FP32, kind="ExternalOutput").ap()

    with tile.TileContext(nc) as tc:
        _build_body(tc, out, hid, enc, ident)
    nc.compile()
    _CACHE["nc"] = nc
    return nc


def make_in_maps(hidden, encoder_outputs):
    hidden = np.ascontiguousarray(np.asarray(hidden, dtype=np.float32))
    enc = np.asarray(encoder_outputs, dtype=np.float32)
    ident = np.eye(PT, dtype=np.float32)
    in_maps = []
    for c in range(NCORES):
        sl = slice(c * BL, (c + 1) * BL)
        in_maps.append({
            "hidden": np.ascontiguousarray(hidden[sl]),
            "encoder_outputs": np.ascontiguousarray(enc[:, sl, :]),
            "identity": ident,
        })
    return in_maps


def kernel(hidden, encoder_outputs, trace=False, **run_kwargs):
    nc = _build()
    in_maps = make_in_maps(hidden, encoder_outputs)
    res = run_bass_kernel_spmd(nc, in_maps, list(range(NCORES)), trace=trace, **run_kwargs)
    out = np.concatenate([r["out"] for r in res.results], axis=0)
    kernel.last_results = res
    return out


# revision 7
# speedup vs baseline: 1.0036x; 1.0036x over previous
"""Bass/Tile TRN2 kernel for nn_Attn: energies = einsum('sbh,bh->sb'), softmax over s,
output attn.T[:, None, :]  ([B, 1, S]).

Sharding: data-parallel over batch B=32 across 8 cores (4 batch elems per core).
Per-core compute:
  - 16 DMA loads of [128, 4*1024] f32 enc tiles (2 MiB each, 16 KiB contiguous per
    partition row) -- DMA-bound at ~358 GB/s per core.
  - For each (s-tile, b): one fused DVE tensor_tensor_reduce:
      prod = enc_tile[:, b] * hid_bcast[b];  energies_col = sum_h(prod)
    (single 1x pass over the data; the product itself lands in a dummy
    broadcast AP and is discarded).
  - energies grid [128, 64] -> PE transpose -> PSUM [64, 128] -> DMA rearrange
    to [4, 2048] (partition = batch).
  - softmax along free dim: reduce_max(negate) -> ACT exp(bias=-max) with fused
    accum sum -> reciprocal -> tensor_scalar mul -> DMA out [4, 1, 2048].
"""

import numpy as np

import concourse.tile as tile
import concourse.mybir as mybir
from concourse import bacc
from concourse.bass_utils import run_bass_kernel_spmd

S, B, H = 2048, 32, 1024
NCORES = 8
BL = B // NCORES  # 4 batch elems per core
PT = 128          # partition tile along s
NST = S // PT     # 16 s-tiles
FP32 = mybir.dt.float32

_CACHE = {}


def _build_body(tc, out, hid, enc, ident):
    nc = tc.nc
    enc_flat = enc.rearrange("s b h -> s (b h)")  # [S, BL*H]

    with (
        tc.tile_pool(name="const", bufs=1) as const_pool,
        tc.tile_pool(name="encp", bufs=5) as enc_pool,
        tc.tile_pool(name="psum", bufs=1, space="PSUM") as psum_pool,
    ):
        ident_sb = const_pool.tile([PT, PT], FP32)
        nc.sync.dma_start(ident_sb[:], ident)

        # hidden broadcast across all 128 partitions: [128, BL*H].
        # One DMA per batch elem so the first multiply only waits for b=0.
        hidb = const_pool.tile([PT, BL * H], FP32)
        for b in range(BL):
            nc.sync.dma_start(
                hidb[:, b * H:(b + 1) * H],
                hid[b:b + 1, :].broadcast_to([PT, H]),
            )

        # energies grid: grid[p, b*NST + st] = energies[st*128 + p, b]
        grid = const_pool.tile([PT, BL * NST], FP32)
        dummy = const_pool.tile([PT, 1], FP32)

        for st in range(NST):
            et = enc_pool.tile([PT, BL * H], FP32, tag="et")
            src = enc_flat[st * PT:(st + 1) * PT, :]
            if st == 0:
                # fine-grained first tile: the first multiply starts after 512 KiB
                for b in range(BL):
                    nc.sync.dma_start(et[:, b * H:(b + 1) * H], src[:, b * H:(b + 1) * H])
            else:
                nc.sync.dma_start(et[:, :2 * H], src[:, :2 * H])
                nc.sync.dma_start(et[:, 2 * H:], src[:, 2 * H:])
            for b in range(BL):
                col = b * NST + st
                # fused multiply + free-dim sum in one DVE pass:
                # out = (et * 1.0) * hidb (discarded), accum = sum(out)
                nc.vector.scalar_tensor_tensor(
                    dummy[:].broadcast_to([PT, H]),
                    et[:, b * H:(b + 1) * H],
                    1.0,
                    hidb[:, b * H:(b + 1) * H],
                    op0=mybir.AluOpType.mult,
                    op1=mybir.AluOpType.mult,
                    accum_out=grid[:, col:col + 1],
                )

        # transpose grid -> [64, 128] in PSUM, then rearrange to [BL, S]
        gridT = psum_pool.tile([BL * NST, PT], FP32)
        nc.tensor.transpose(gridT[:], grid[:], ident_sb[:])

        gridT_sb = const_pool.tile([BL * NST, PT], FP32)
        nc.scalar.copy(gridT_sb[:], gridT[:])

        # Gather the [64, 128] rows into [4, 2048] (row b <- partitions b*16..b*16+15).
        # One DMA: src/dst element orders match (b, t, p).
        eT = const_pool.tile([BL, S], FP32)
        nc.sync.dma_start(eT[:, :], gridT_sb[:, :])

        # softmax over free dim (s), per-partition (b)
        negm = const_pool.tile([BL, 1], FP32)
        nc.vector.reduce_max(negm[:], eT[:], axis=mybir.AxisListType.X, negate=True)

        p_t = const_pool.tile([BL, S], FP32)
        ssum = const_pool.tile([BL, 1], FP32)
        nc.scalar.activation(
            p_t[:], eT[:], mybir.ActivationFunctionType.Exp,
            bias=negm[:], scale=1.0, accum_out=ssum[:],
        )

        rsum = const_pool.tile([BL, 1], FP32)
        nc.vector.reciprocal(rsum[:], ssum[:])

        attn = const_pool.tile([BL, S], FP32)
        nc.vector.tensor_scalar_mul(attn[:], p_t[:], rsum[:])

        nc.sync.dma_start(out.rearrange("b o s -> b (o s)"), attn[:])


def _build():
    if "nc" in _CACHE:
        return _CACHE["nc"]
    nc = bacc.Bacc(
        "TRN2",
        target_bir_lowering=False,
        debug=False,
        enable_asserts=False,
        num_devices=NCORES,
    )
    hid = nc.dram_tensor("hidden", [BL, H], FP32, kind="ExternalInput").ap()
    enc = nc.dram_tensor("encoder_outputs", [S, BL, H], FP32, kind="ExternalInput").ap()
    ident = nc.dram_tensor("identity", [PT, PT], FP32, kind="ExternalInput").ap()
    out = nc.dram_tensor("out", [BL, 1, S], FP32, kind="ExternalOutput").ap()

    with tile.TileContext(nc) as tc:
        _build_body(tc, out, hid, enc, ident)
    nc.compile()
    _CACHE["nc"] = nc
    return nc


def make_in_maps(hidden, encoder_outputs):
    hidden = np.ascontiguousarray(np.asarray(hidden, dtype=np.float32))
    enc = np.asarray(encoder_outputs, dtype=np.float32)
    ident = np.eye(PT, dtype=np.float32)
    in_maps = []
    for c in range(NCORES):
        sl = slice(c * BL, (c + 1) * BL)
        in_maps.append({
            "hidden": np.ascontiguousarray(hidden[sl]),
            "encoder_outputs": np.ascontiguousarray(enc[:, sl, :]),
            "identity": ident,
        })
    return in_maps


def kernel(hidden, encoder_outputs, trace=False, **run_kwargs):
    nc = _build()
    in_maps = make_in_maps(hidden, encoder_outputs)
    res = run_bass_kernel_spmd(nc, in_maps, list(range(NCORES)), trace=trace, **run_kwargs)
    out = np.concatenate([r["out"] for r in res.results], axis=0)
    kernel.last_results = res
    return out


# revision 8
# speedup vs baseline: 1.0224x; 1.0188x over previous
"""Bass/Tile TRN2 kernel for nn_Attn: energies = einsum('sbh,bh->sb'), softmax over s,
output attn.T[:, None, :]  ([B, 1, S]).

Sharding: data-parallel over batch B=32 across 8 cores (4 batch elems per core).
Per-core compute:
  - 16 DMA loads of [128, 4*1024] f32 enc tiles (2 MiB each, 16 KiB contiguous per
    partition row) -- DMA-bound at ~358 GB/s per core.
  - For each (s-tile, b): one fused DVE tensor_tensor_reduce:
      prod = enc_tile[:, b] * hid_bcast[b];  energies_col = sum_h(prod)
    (single 1x pass over the data; the product itself lands in a dummy
    broadcast AP and is discarded).
  - energies grid [128, 64] -> PE transpose -> PSUM [64, 128] -> DMA rearrange
    to [4, 2048] (partition = batch).
  - softmax along free dim: reduce_max(negate) -> ACT exp(bias=-max) with fused
    accum sum -> reciprocal -> tensor_scalar mul -> DMA out [4, 1, 2048].
"""

import numpy as np

import concourse.tile as tile
import concourse.mybir as mybir
from concourse import bacc
from concourse.bass_utils import run_bass_kernel_spmd

S, B, H = 2048, 32, 1024
NCORES = 8
BL = B // NCORES  # 4 batch elems per core
PT = 128          # partition tile along s
NST = S // PT     # 16 s-tiles
FP32 = mybir.dt.float32

_CACHE = {}


def _build_body(tc, out, hid, enc, ident):
    nc = tc.nc
    enc_flat = enc.rearrange("s b h -> s (b h)")  # [S, BL*H]

    with (
        tc.tile_pool(name="const", bufs=1) as const_pool,
        tc.tile_pool(name="encp", bufs=4) as enc_pool,
        tc.tile_pool(name="psum", bufs=1, space="PSUM") as psum_pool,
    ):
        # hidden as one row, then GpSimd partition-broadcast to 128 partitions
        # (per-b so the first multiply only waits for b=0's slice).
        hid_row = const_pool.tile([1, BL * H], FP32)
        nc.sync.dma_start(hid_row[0:1, :], hid.rearrange("b h -> (b h)").unsqueeze(0))
        ident_sb = const_pool.tile([PT, PT], FP32)
        nc.sync.dma_start(ident_sb[:], ident)

        hidb = const_pool.tile([PT, BL * H], FP32)
        for b in range(BL):
            nc.gpsimd.partition_broadcast(
                hidb[:, b * H:(b + 1) * H], hid_row[0:1, b * H:(b + 1) * H]
            )

        # energies grid: grid[p, st*BL + b] = energies[st*128 + p, b]
        grid = const_pool.tile([PT, BL * NST], FP32)
        dummy = const_pool.tile([PT, 1], FP32)

        # energies^T accumulates in PSUM: eTp[b, st*128 + p]
        eTp = psum_pool.tile([BL, S], FP32)

        for st in range(NST):
            et = enc_pool.tile([PT, BL * H], FP32, tag="et")
            src = enc_flat[st * PT:(st + 1) * PT, :]
            if st == 0:
                # fine-grained first tile: the first multiply starts after 512 KiB
                for b in range(BL):
                    nc.sync.dma_start(et[:, b * H:(b + 1) * H], src[:, b * H:(b + 1) * H])
            else:
                nc.sync.dma_start(et[:, :2 * H], src[:, :2 * H])
                nc.sync.dma_start(et[:, 2 * H:], src[:, 2 * H:])
            for b in range(BL):
                col = st * BL + b
                # fused multiply + free-dim sum in one DVE pass:
                # out = (et * 1.0) * hidb (discarded), accum = sum(out)
                nc.vector.scalar_tensor_tensor(
                    dummy[:].broadcast_to([PT, H]),
                    et[:, b * H:(b + 1) * H],
                    1.0,
                    hidb[:, b * H:(b + 1) * H],
                    op0=mybir.AluOpType.mult,
                    op1=mybir.AluOpType.mult,
                    accum_out=grid[:, col:col + 1],
                )
            # transpose this s-tile's [128, 4] energies into eTp[:, st*128:...]
            # (runs on the otherwise-idle PE, hidden under the DMA/DVE stream)
            nc.tensor.transpose(
                eTp[:, st * PT:(st + 1) * PT],
                grid[:, st * BL:(st + 1) * BL],
                ident_sb[:],
            )

        # softmax over free dim (s), per-partition (b), reading energies^T from PSUM
        negm = const_pool.tile([BL, 1], FP32)
        nc.vector.reduce_max(negm[:], eTp[:], axis=mybir.AxisListType.X, negate=True)

        p_t = const_pool.tile([BL, S], FP32)
        ssum = const_pool.tile([BL, 1], FP32)
        nc.scalar.activation(
            p_t[:], eTp[:], mybir.ActivationFunctionType.Exp,
            bias=negm[:], scale=1.0, accum_out=ssum[:],
        )

        rsum = const_pool.tile([BL, 1], FP32)
        nc.vector.reciprocal(rsum[:], ssum[:])

        # scale + store in two halves so the first store overlaps the second mul
        attn = const_pool.tile([BL, S], FP32)
        out_flat = out.rearrange("b o s -> b (o s)")
        nc.vector.tensor_scalar_mul(attn[:, :S // 2], p_t[:, :S // 2], rsum[:])
        nc.sync.dma_start(out_flat[:, :S // 2], attn[:, :S // 2])
        nc.vector.tensor_scalar_mul(attn[:, S // 2:], p_t[:, S // 2:], rsum[:])
        nc.sync.dma_start(out_flat[:, S // 2:], attn[:, S // 2:])


def _build():
    if "nc" in _CACHE:
        return _CACHE["nc"]
    nc = bacc.Bacc(
        "TRN2",
        target_bir_lowering=False,
        debug=False,
        enable_asserts=False,
        num_devices=NCORES,
    )
    hid = nc.dram_tensor("hidden", [BL, H], FP32, kind="ExternalInput").ap()
    enc = nc.dram_tensor("encoder_outputs", [S, BL, H], FP32, kind="ExternalInput").ap()
    ident = nc.dram_tensor("identity", [PT, PT], FP32, kind="ExternalInput").ap()
    out = nc.dram_tensor("out", [BL, 1, S], FP32, kind="ExternalOutput").ap()

    with tile.TileContext(nc) as tc:
        _build_body(tc, out, hid, enc, ident)
    nc.compile()
    _CACHE["nc"] = nc
    return nc


def make_in_maps(hidden, encoder_outputs):
    hidden = np.ascontiguousarray(np.asarray(hidden, dtype=np.float32))
    enc = np.asarray(encoder_outputs, dtype=np.float32)
    ident = np.eye(PT, dtype=np.float32)
    in_maps = []
    for c in range(NCORES):
        sl = slice(c * BL, (c + 1) * BL)
        in_maps.append({
            "hidden": np.ascontiguousarray(hidden[sl]),
            "encoder_outputs": np.ascontiguousarray(enc[:, sl, :]),
            "identity": ident,
        })
    return in_maps


def kernel(hidden, encoder_outputs, trace=False, **run_kwargs):
    nc = _build()
    in_maps = make_in_maps(hidden, encoder_outputs)
    res = run_bass_kernel_spmd(nc, in_maps, list(range(NCORES)), trace=trace, **run_kwargs)
    out = np.concatenate([r["out"] for r in res.results], axis=0)
    kernel.last_results = res
    return out
